# revision 38
# baseline (speedup 1.0000x reference)
"""DKD keypoint-detection kernel for Trainium2 (Bass/Tile), batch-parallel on 8 cores.

Per core (one image [1,480,640] scores + [64,480,640] desc):
  NMS (5 maxpools, ping-pong layouts via PE transpose) -> 3x3 block-reduce with
  argmax carry -> qkey (value mantissa | 13-bit position code) -> per-partition
  top-32 (max8/match_replace) -> kth_largest prune -> sparse_gather compaction
  -> exact lex rank (value desc, pixel idx asc) by counting -> rank scatter ->
  span-gather patches + softmax refinement -> bilinear grid-sample of scores
  and descriptors (bf16 transposed desc copy built on device, overlapped with
  NMS) -> L2 normalize.  Outputs kxy [500,2], desc [500,64], kpt [500],
  disp [500].

HW notes baked in: indirect DMA supports ONE index per partition-row only;
DVE int arithmetic runs through the fp32 ALU (exact only below 2^24);
f32->i32 cast rounds to nearest (not trunc).
"""
import sys

sys.path.insert(0, "/opt/trn_rl_repo")
import numpy as np

import concourse.bass as bass
import concourse.bacc as bacc
import concourse.mybir as mybir
import concourse.tile as tile
from concourse.bass_utils import run_bass_kernel_spmd

dt = mybir.dt
Alu = mybir.AluOpType
ActF = mybir.ActivationFunctionType
Ax = mybir.AxisListType

H, W = 480, 640
HW = H * W
HP = 120          # rows per chunk (4 chunks) in A layout
NBW = 213         # 3-pixel block columns (col 639 always zero)
NJ = 426          # candidate slots per partition = 2 (hb) * 213 (wb)
NKEEP = 510       # prune target: t = NKEEP-th largest qkey
NSLOT = 512
TOPK = 500
C1 = float(0.5 - 2.0 ** -15)  # round-to-nearest -> floor helper
DCH = 150         # desc transpose chunks of 2048 pixels


def _consts():
    c = {}
    c["ident"] = np.eye(128, dtype=np.float32)
    c["identb"] = np.eye(128, dtype=np.float32)
    c["ones1"] = np.ones((1, 128), np.float32)
    j = np.arange(NJ)
    hb, wb = j // NBW, j % NBW
    c["jor_e"] = np.tile((511 - (hb * 256 + wb)).astype(np.int32), (128, 1))
    i3 = np.tile(np.repeat(np.arange(3, dtype=np.float32), 3), 4)   # col offset
    r3 = np.tile(np.tile(np.arange(3, dtype=np.float32), 3), 4)    # row offset
    c["cc36"] = np.tile(i3, (128, 1))
    c["rr36"] = np.tile(r3, (128, 1))
    c["p3"] = (3.0 * np.arange(128, dtype=np.float32))[:, None]
    col = np.arange(128)[:, None] + 128 * np.arange(5)[None, :]
    cm = ((col >= 3) & (col <= 637)).astype(np.float32)
    c["colmask"] = cm.reshape(128, 5, 1).copy()
    dx = np.tile(np.tile(np.arange(5, dtype=np.float32) - 2, 5), 4)
    dy = np.tile(np.repeat(np.arange(5, dtype=np.float32) - 2, 5), 4)
    c["dxr"] = np.tile(dx, (128, 1))
    c["dyr"] = np.tile(dy, (128, 1))
    return c


CONSTS = _consts()
_CONST_DT = {"jor_e": dt.int32}


def _mp_h(nc, sb, src, dst):
    """5-tap max along cols (2..641) of [120,4,648] A-layout tiles."""
    h4 = sb.tile([HP, 4, 648], dt.float32, tag="mp_h4")
    nc.vector.tensor_tensor(
        out=dst[:, :, 0:646], in0=src[:, :, 0:646], in1=src[:, :, 1:647], op=Alu.max
    )
    nc.vector.tensor_tensor(
        out=h4[:, :, 0:644], in0=dst[:, :, 0:644], in1=dst[:, :, 2:646], op=Alu.max
    )
    nc.vector.tensor_tensor(
        out=dst[:, :, 2:642], in0=h4[:, :, 0:640], in1=src[:, :, 4:644], op=Alu.max
    )


def _mp_v(nc, sb, src, dst):
    """5-tap max along h (2..481) of [128,5,484] B-layout tiles."""
    v4 = sb.tile([128, 5, 484], dt.float32, tag="mp_v4")
    nc.vector.tensor_tensor(
        out=dst[:, :, 0:483], in0=src[:, :, 0:483], in1=src[:, :, 1:484], op=Alu.max
    )
    nc.vector.tensor_tensor(
        out=v4[:, :, 0:481], in0=dst[:, :, 0:481], in1=dst[:, :, 2:483], op=Alu.max
    )
    nc.vector.tensor_tensor(
        out=dst[:, :, 2:482], in0=v4[:, :, 0:480], in1=src[:, :, 4:484], op=Alu.max
    )


def _t_a2b(nc, ps, ident, src, dst):
    """Transpose A[120,4,648](cols 2..641) -> B[128,5,484](h 2..481)."""
    tp = ps.tile([128, 5, 512], dt.float32, tag="ps_big")
    for c in range(4):
        for w in range(5):
            nc.tensor.transpose(
                out=tp[:, w, c * HP : (c + 1) * HP],
                in_=src[0:HP, c, 2 + 128 * w : 2 + 128 * (w + 1)],
                identity=ident[0:HP, 0:HP],
            )
    nc.scalar.copy(out=dst[:, :, 2:482], in_=tp[:, :, 0:480])


def _t_b2a(nc, ps, ident, src, dst):
    """Transpose B[128,5,484](h 2..481) -> A[120,4,648](cols 2..641)."""
    tp = ps.tile([HP, 4, 5, 128], dt.float32, tag="ps_big")
    for c in range(4):
        for w in range(5):
            nc.tensor.transpose(
                out=tp[:, c, w, :],
                in_=src[0:128, w, 2 + HP * c : 2 + HP * (c + 1)],
                identity=ident[0:128, 0:128],
            )
    nc.scalar.copy(
        out=dst[:, :, 2:642], in_=tp[:].rearrange("p c w q -> p c (w q)")
    )


def _sview3(t, n_mid, stride_mid, n_in):
    """[128, F] tile -> AP [128, n_mid, n_in] with mid-stride over free dim."""
    base = t[:]
    return bass.AP(base.tensor, base.offset, [list(base.ap[0]), [stride_mid, n_mid], [1, n_in]])


def build_nc(dbg=False):
    nc = bacc.Bacc(None, target_bir_lowering=False)
    scores = nc.dram_tensor("scores", [H, W], dt.float32, kind="ExternalInput")
    descf = nc.dram_tensor("descf", [64 * HW], dt.float32, kind="ExternalInput")
    cst = {}
    for k, v in CONSTS.items():
        cst[k] = nc.dram_tensor(
            k, list(v.shape), _CONST_DT.get(k, dt.float32), kind="ExternalInput"
        )

    o_kxy = nc.dram_tensor("out_kxy", [TOPK, 2], dt.float32, kind="ExternalOutput")
    o_desc = nc.dram_tensor("out_desc", [TOPK, 64], dt.float32, kind="ExternalOutput")
    o_kpt = nc.dram_tensor("out_kpt", [TOPK], dt.float32, kind="ExternalOutput")
    o_disp = nc.dram_tensor("out_disp", [TOPK], dt.float32, kind="ExternalOutput")

    kw = dict(kind="ExternalOutput") if dbg else {}
    nmsd = nc.dram_tensor("nmsd", [W * H], dt.float32, **kw)  # col-major nms
    vseld = nc.dram_tensor("vseld", [128 * 32], dt.float32, **kw)
    vseld2 = nc.dram_tensor("vseld2", [128 * 32], dt.float32, **kw)
    sgd = nc.dram_tensor("sgd", [NSLOT], dt.float32, **kw)
    sgd2 = nc.dram_tensor("sgd2", [NSLOT], dt.float32, **kw)
    vvd = nc.dram_tensor("vvd", [NSLOT], dt.float32, **kw)
    pixd = nc.dram_tensor("pixd", [NSLOT], dt.float32, **kw)
    kpd = nc.dram_tensor("kpd", [NSLOT], dt.float32, **kw)
    descT = nc.dram_tensor("descT", [HW * 64], dt.bfloat16)

    with tile.TileContext(nc) as tc:
        with tc.tile_pool(name="sb", bufs=1) as sb, tc.tile_pool(
            name="ps", bufs=1, space="PSUM"
        ) as ps, tc.tile_pool(name="dsb", bufs=3) as dsb, tc.tile_pool(
            name="dps", bufs=2, space="PSUM"
        ) as dps:
            ident = sb.tile([128, 128], dt.float32, tag="ident")
            identb = sb.tile([128, 128], dt.bfloat16, tag="identb")
            ones1 = sb.tile([1, 128], dt.float32, tag="ones1")
            nc.sync.dma_start(ident[:], cst["ident"][:])
            nc.gpsimd.dma_start(identb[:], cst["identb"][:])  # f32->bf16 cast DMA
            nc.sync.dma_start(ones1[:], cst["ones1"][:])

            # ===== descriptor transpose to [HW, 64] bf16 (overlappable) =====
            dview = descf[:].rearrange("(c x) -> c x", c=64)
            tview = descT[:].rearrange("(x c) -> x c", c=64)
            for ci in range(DCH):
                x0 = 2048 * ci
                stg = dsb.tile([64, 2048], dt.bfloat16, tag="dstg")
                nc.gpsimd.dma_start(stg[:], dview[:, x0 : x0 + 2048])
                dout = dsb.tile([128, 16, 64], dt.bfloat16, tag="dout")
                for half in range(2):
                    pt = dps.tile([128, 8, 64], dt.bfloat16, tag="ps_d")
                    for b in range(8):
                        bb = half * 8 + b
                        nc.tensor.transpose(
                            out=pt[:, b, :],
                            in_=stg[0:64, 128 * bb : 128 * (bb + 1)],
                            identity=identb[0:64, 0:64],
                        )
                    if (ci + half) % 2 == 0:
                        nc.vector.tensor_copy(
                            dout[:, half * 8 : half * 8 + 8, :], pt[:]
                        )
                    else:
                        nc.scalar.copy(
                            out=dout[:, half * 8 : half * 8 + 8, :], in_=pt[:]
                        )
                nc.sync.dma_start(
                    tview[x0 : x0 + 2048, :].rearrange("(b p) c -> p b c", p=128),
                    dout[:],
                )

            # ========================= NMS ==================================
            def A(tag):
                t = sb.tile([HP, 4, 648], dt.float32, tag=tag, name=tag)
                return t

            def B(tag):
                t = sb.tile([128, 5, 484], dt.float32, tag=tag, name=tag)
                return t

            X = A("X")
            nc.vector.memset(X[:, :, 0:2], 0.0)
            nc.vector.memset(X[:, :, 642:648], 0.0)
            nc.sync.dma_start(
                X[:, :, 2:642], scores[:].rearrange("(c p) w -> p c w", p=HP)
            )
            sT = B("sT")
            _t_a2b(nc, ps, ident, X, sT)

            HH = A("HH")
            _mp_h(nc, sb, X, HH)
            HB = B("HB")
            nc.vector.memset(HB[:, :, 0:2], 0.0)
            nc.vector.memset(HB[:, :, 482:484], 0.0)
            _t_a2b(nc, ps, ident, HH, HB)
            MP = B("MP")
            _mp_v(nc, sb, HB, MP)

            M1 = B("M1")
            nc.vector.memset(M1[:, :, 0:2], 0.0)
            nc.vector.memset(M1[:, :, 482:484], 0.0)
            nc.vector.tensor_tensor(
                out=M1[:, :, 2:482], in0=sT[:, :, 2:482], in1=MP[:, :, 2:482],
                op=Alu.is_equal,
            )

            SS = A("SS")
            nc.vector.memset(SS[:, :, 0:2], 0.0)
            nc.vector.memset(SS[:, :, 642:648], 0.0)
            SSB = B("SSB")
            D_A = A("D_A")
            D_B = B("D_B")
            NEW = B("NEW")

            for it in range(2):
                VD = sb.tile([128, 5, 484], dt.float32, tag="HB")
                _mp_v(nc, sb, M1, VD)
                TD = sb.tile([HP, 4, 648], dt.float32, tag="TD")
                nc.vector.memset(TD[:, :, 0:2], 0.0)
                nc.vector.memset(TD[:, :, 642:648], 0.0)
                _t_b2a(nc, ps, ident, VD, TD)
                _mp_h(nc, sb, TD, D_A)
                _t_a2b(nc, ps, ident, D_A, D_B)
                nc.vector.tensor_scalar(
                    out=D_A[:, :, 2:642], in0=D_A[:, :, 2:642],
                    scalar1=0.0, scalar2=None, op0=Alu.is_gt,
                )
                nc.vector.tensor_scalar(
                    out=D_B[:, :, 2:482], in0=D_B[:, :, 2:482],
                    scalar1=0.0, scalar2=None, op0=Alu.is_gt,
                )
                nc.vector.tensor_scalar(
                    out=SS[:, :, 2:642], in0=D_A[:, :, 2:642],
                    scalar1=-1.0, scalar2=1.0, op0=Alu.mult, op1=Alu.add,
                )
                nc.vector.tensor_tensor(
                    out=SS[:, :, 2:642], in0=SS[:, :, 2:642], in1=X[:, :, 2:642],
                    op=Alu.mult,
                )
                nc.vector.tensor_scalar(
                    out=SSB[:, :, 2:482], in0=D_B[:, :, 2:482],
                    scalar1=-1.0, scalar2=1.0, op0=Alu.mult, op1=Alu.add,
                )
                nc.vector.tensor_tensor(
                    out=SSB[:, :, 2:482], in0=SSB[:, :, 2:482], in1=sT[:, :, 2:482],
                    op=Alu.mult,
                )
                HH2 = sb.tile([HP, 4, 648], dt.float32, tag="HH")
                _mp_h(nc, sb, SS, HH2)
                HB2 = sb.tile([128, 5, 484], dt.float32, tag="HB")
                nc.vector.memset(HB2[:, :, 0:2], 0.0)
                nc.vector.memset(HB2[:, :, 482:484], 0.0)
                _t_a2b(nc, ps, ident, HH2, HB2)
                _mp_v(nc, sb, HB2, MP)
                nc.vector.tensor_tensor(
                    out=NEW[:, :, 2:482], in0=SSB[:, :, 2:482], in1=MP[:, :, 2:482],
                    op=Alu.is_equal,
                )
                nc.vector.scalar_tensor_tensor(
                    out=NEW[:, :, 2:482], in0=D_B[:, :, 2:482], scalar=-1.0,
                    op0=Alu.mult, in1=NEW[:, :, 2:482], op1=Alu.add,
                )
                nc.vector.tensor_scalar(
                    out=NEW[:, :, 2:482], in0=NEW[:, :, 2:482],
                    scalar1=0.0, scalar2=None, op0=Alu.is_gt,
                )
                nc.vector.tensor_tensor(
                    out=M1[:, :, 2:482], in0=M1[:, :, 2:482], in1=NEW[:, :, 2:482],
                    op=Alu.max,
                )

            NMS = sb.tile([128, 5, 484], dt.float32, tag="SSB")
            nc.vector.tensor_tensor(
                out=NMS[:, :, 2:482], in0=sT[:, :, 2:482], in1=M1[:, :, 2:482],
                op=Alu.mult,
            )
            cmk = sb.tile([128, 5, 1], dt.float32, tag="cmk")
            nc.sync.dma_start(cmk[:], cst["colmask"][:])
            nc.vector.tensor_tensor(
                out=NMS[:, :, 2:482], in0=NMS[:, :, 2:482],
                in1=cmk[:].to_broadcast([128, 5, 480]), op=Alu.mult,
            )
            nc.vector.memset(NMS[:, :, 2:5], 0.0)
            nc.vector.memset(NMS[:, :, 480:482], 0.0)
            nc.sync.dma_start(
                nmsd[:].rearrange("(w p h) -> p w h", p=128, h=H), NMS[:, :, 2:482]
            )

            # ============ 3x3 block reduce with argmax carry ================
            a0 = NMS[:, :, 2:482:3]
            a1 = NMS[:, :, 3:482:3]
            a2 = NMS[:, :, 4:482:3]
            HBK = sb.tile([128, 5, 160], dt.float32, tag="HBK")
            nc.vector.tensor_tensor(out=HBK[:], in0=a0, in1=a1, op=Alu.max)
            nc.vector.tensor_tensor(out=HBK[:], in0=HBK[:], in1=a2, op=Alu.max)
            CS = sb.tile([128, 2, 640], dt.float32, tag="CS")
            ctp = ps.tile([128, 2, 5, 128], dt.float32, tag="ps_big")
            for w in range(5):
                nc.tensor.transpose(
                    out=ctp[:, 0, w, :], in_=HBK[0:128, w, 0:128],
                    identity=ident[0:128, 0:128],
                )
                nc.tensor.transpose(
                    out=ctp[0:32, 1, w, :], in_=HBK[0:128, w, 128:160],
                    identity=ident[0:128, 0:128],
                )
            nc.vector.memset(CS[32:64, 1, :], 0.0)
            nc.vector.memset(CS[64:128, 1, :], 0.0)
            nc.scalar.copy(
                out=CS[:, 0, :], in_=ctp[:, 0, :, :].rearrange("p w q -> p (w q)")
            )
            nc.scalar.copy(
                out=CS[0:32, 1, :],
                in_=ctp[0:32, 1, :, :].rearrange("p w q -> p (w q)"),
            )
            C = sb.tile([128, 2, NBW], dt.float32, tag="C")
            nc.vector.tensor_tensor(
                out=C[:], in0=CS[:, :, 0:639:3], in1=CS[:, :, 1:639:3], op=Alu.max
            )
            nc.vector.tensor_tensor(
                out=C[:], in0=C[:], in1=CS[:, :, 2:639:3], op=Alu.max
            )
            jor = sb.tile([128, NJ], dt.int32, tag="jor")
            nc.sync.dma_start(jor[:], cst["jor_e"][:])
            QK = sb.tile([128, NJ], dt.int32, tag="QK")
            QKA = sb.tile([128, NJ], dt.int32, tag="QKA")
            nc.vector.tensor_scalar(
                out=QKA[:],
                in0=C[:].rearrange("p a b -> p (a b)").bitcast(dt.int32),
                scalar1=~0x1FF, scalar2=None, op0=Alu.bitwise_and,
            )
            nc.vector.tensor_tensor(
                out=QK[:], in0=QKA[:], in1=jor[:], op=Alu.bitwise_or
            )
            QKf = QK[:].bitcast(dt.float32)

            # ---- per-partition top-32 ----
            V = sb.tile([128, 32], dt.float32, tag="V")
            for r in range(4):
                nc.vector.max(out=V[:, r * 8 : (r + 1) * 8], in_=QKf)
                nc.vector.match_replace(
                    out=QKf, in_to_replace=V[:, r * 8 : (r + 1) * 8],
                    in_values=QKf, imm_value=-1.0,
                )

            # ---- kth_largest -> threshold broadcast ----
            ko = sb.tile([1, 2], dt.float32, tag="ko")
            q = 1.0 - (NKEEP - 1.5) / (128 * 32 - 1)
            nc.gpsimd.kth_largest(
                out_ap=ko[:], in_ap=V[:], n_per_lane=32, k=NKEEP - 1, quantile=q
            )
            tb_ps = ps.tile([128, 1], dt.float32, tag="ps_rep")
            nc.tensor.matmul(tb_ps[:], lhsT=ones1[0:1, 0:128], rhs=ko[0:1, 1:2])
            tb = sb.tile([128, 1], dt.float32, tag="tb")
            nc.vector.tensor_copy(tb[:], tb_ps[:])

            # ---- compaction via sparse_gather ----
            MM = sb.tile([128, 32], dt.int32, tag="MM")
            VS = sb.tile([128, 32], dt.float32, tag="VS")
            nc.vector.tensor_scalar(
                out=MM[:], in0=V[:], scalar1=tb[:, 0:1], scalar2=None, op0=Alu.is_ge
            )
            p3 = sb.tile([128, 1], dt.float32, tag="p3")
            nc.sync.dma_start(p3[:], cst["p3"][:])
            nc.vector.memset(VS[:], -1.0)
            nc.vector.copy_predicated(out=VS[:], mask=MM[:], data=V[:])
            nc.sync.dma_start(vseld[:].rearrange("(p i) -> p i", p=128), VS[:])
            VS2 = sb.tile([128, 32], dt.float32, tag="VS2")
            nc.vector.memset(VS2[:], -1.0)
            nc.vector.copy_predicated(
                out=VS2[:], mask=MM[:], data=p3[:].to_broadcast([128, 32])
            )
            nc.sync.dma_start(vseld2[:].rearrange("(p i) -> p i", p=128), VS2[:])
            W16 = sb.tile([16, 256], dt.float32, tag="W16")
            SGO = sb.tile([16, 32], dt.float32, tag="SGO")
            nf = sb.tile([1, 1], dt.uint32, tag="nf")
            CQ = sb.tile([128, 4], dt.float32, tag="CQ")
            CP3 = sb.tile([128, 4], dt.float32, tag="CP3")
            for src_d, dst_d, dst_t in ((vseld, sgd, CQ), (vseld2, sgd2, CP3)):
                nc.sync.dma_start(W16[:], src_d[:].rearrange("(f q) -> q f", q=16))
                nc.vector.memset(SGO[:], -1.0)
                nc.gpsimd.sparse_gather(out=SGO[:], in_=W16[:], num_found=nf[:])
                nc.sync.dma_start(dst_d[:].rearrange("(f q) -> q f", q=16), SGO[:])
                nc.sync.dma_start(
                    dst_t[:], dst_d[:].rearrange("(s p) -> p s", p=128)
                )

            # ---- decode compacted qkeys ----
            KM = sb.tile([128, 4], dt.float32, tag="KM")
            nc.vector.tensor_scalar(
                out=KM[:], in0=CQ[:], scalar1=0.0, scalar2=None, op0=Alu.is_ge
            )
            E9 = sb.tile([128, 4], dt.int32, tag="E9")
            nc.vector.tensor_scalar(
                out=E9[:], in0=CQ[:].bitcast(dt.int32), scalar1=0x1FF,
                scalar2=None, op0=Alu.bitwise_and,
            )
            nc.vector.tensor_scalar(
                out=E9[:], in0=E9[:], scalar1=-1, scalar2=511,
                op0=Alu.mult, op1=Alu.add,
            )
            HBI = sb.tile([128, 4], dt.int32, tag="HBI")
            WBI = sb.tile([128, 4], dt.int32, tag="WBI")
            nc.vector.tensor_scalar(
                out=HBI[:], in0=E9[:], scalar1=8, scalar2=None,
                op0=Alu.logical_shift_right,
            )
            nc.vector.tensor_scalar(
                out=WBI[:], in0=E9[:], scalar1=255, scalar2=None, op0=Alu.bitwise_and
            )
            HBF = sb.tile([128, 4], dt.float32, tag="HBF")
            WBF = sb.tile([128, 4], dt.float32, tag="WBF")
            nc.vector.tensor_copy(HBF[:], HBI[:])
            nc.vector.tensor_copy(WBF[:], WBI[:])
            PY0 = sb.tile([128, 4], dt.float32, tag="PY0")
            PX0 = sb.tile([128, 4], dt.float32, tag="PX0")
            nc.vector.scalar_tensor_tensor(
                out=PY0[:], in0=HBF[:], scalar=384.0, op0=Alu.mult,
                in1=CP3[:], op1=Alu.add,
            )
            nc.vector.tensor_scalar(
                out=PX0[:], in0=WBF[:], scalar1=3.0, scalar2=None, op0=Alu.mult
            )
            # block span (col-major nms): 2 cols + 3 = 963 elems from px0*480+py0
            SPI = sb.tile([128, 4], dt.float32, tag="SPI")
            nc.vector.scalar_tensor_tensor(
                out=SPI[:], in0=PX0[:], scalar=480.0, op0=Alu.mult, in1=PY0[:],
                op1=Alu.add,
            )
            nc.vector.tensor_scalar(
                out=SPI[:], in0=SPI[:], scalar1=0.0, scalar2=float(HW - 963),
                op0=Alu.max, op1=Alu.min,
            )
            SPII = sb.tile([128, 4], dt.int32, tag="SPII")
            nc.vector.tensor_copy(SPII[:], SPI[:])
            B9 = sb.tile([128, 4, 9], dt.float32, tag="B9")
            nview = nmsd[:].rearrange("(n o) -> n o", o=1)
            for s in range(4):
                SPN9 = sb.tile([128, 963], dt.float32, tag="SS")
                nc.gpsimd.indirect_dma_start(
                    out=SPN9[:], out_offset=None, in_=nview,
                    in_offset=bass.IndirectOffsetOnAxis(ap=SPII[:, s : s + 1], axis=0),
                )
                nc.vector.tensor_copy(
                    B9[:, s, :].rearrange("p (i r) -> p i r", i=3),
                    _sview3(SPN9, 3, 480, 3),
                )
            VV = sb.tile([128, 4], dt.float32, tag="VV")
            nc.vector.tensor_reduce(out=VV[:], in_=B9[:], axis=Ax.X, op=Alu.max)
            nc.vector.tensor_tensor(out=VV[:], in0=VV[:], in1=KM[:], op=Alu.mult)
            EQ = sb.tile([128, 36], dt.float32, tag="EQ")
            nc.vector.tensor_tensor(
                out=EQ[:].rearrange("p (c k) -> p c k", c=4),
                in0=B9[:], in1=VV[:].to_broadcast([128, 4, 9]), op=Alu.is_equal,
            )
            rr36 = sb.tile([128, 36], dt.float32, tag="rr36")
            cc36 = sb.tile([128, 36], dt.float32, tag="cc36")
            nc.sync.dma_start(rr36[:], cst["rr36"][:])
            nc.sync.dma_start(cc36[:], cst["cc36"][:])
            T36 = sb.tile([128, 36], dt.float32, tag="T36")
            PY = sb.tile([128, 4], dt.float32, tag="PY")
            PX = sb.tile([128, 4], dt.float32, tag="PX")
            nc.vector.tensor_tensor(out=T36[:], in0=EQ[:], in1=rr36[:], op=Alu.mult)
            nc.vector.tensor_reduce(
                out=PY[:], in_=T36[:].rearrange("p (c k) -> p c k", c=4),
                axis=Ax.X, op=Alu.add,
            )
            nc.vector.tensor_tensor(out=PY[:], in0=PY[:], in1=PY0[:], op=Alu.add)
            nc.vector.tensor_tensor(out=T36[:], in0=EQ[:], in1=cc36[:], op=Alu.mult)
            nc.vector.tensor_reduce(
                out=PX[:], in_=T36[:].rearrange("p (c k) -> p c k", c=4),
                axis=Ax.X, op=Alu.add,
            )
            nc.vector.tensor_tensor(out=PX[:], in0=PX[:], in1=PX0[:], op=Alu.add)
            PIX = sb.tile([128, 4], dt.float32, tag="PIX")
            nc.vector.scalar_tensor_tensor(
                out=PIX[:], in0=PY[:], scalar=640.0, op0=Alu.mult, in1=PX[:],
                op1=Alu.add,
            )

            # ---- replicate (VV, PIX); exact lex rank by counting ----
            nc.sync.dma_start(vvd[:].rearrange("(s p) -> p s", p=128), VV[:])
            nc.sync.dma_start(pixd[:].rearrange("(s p) -> p s", p=128), PIX[:])
            FV = sb.tile([1, NSLOT], dt.float32, tag="f1")
            FI = sb.tile([1, NSLOT], dt.float32, tag="f2")
            nc.sync.dma_start(FV[:], vvd[:].rearrange("(o n) -> o n", o=1))
            nc.sync.dma_start(FI[:], pixd[:].rearrange("(o n) -> o n", o=1))
            rv_ps = ps.tile([128, NSLOT], dt.float32, tag="ps_rep")
            nc.tensor.matmul(rv_ps[:], lhsT=ones1[0:1, 0:128], rhs=FV[0:1, :])
            RPV = sb.tile([128, NSLOT], dt.float32, tag="D_A")
            nc.scalar.copy(out=RPV[:], in_=rv_ps[:])
            ri_ps = ps.tile([128, NSLOT], dt.float32, tag="ps_rep")
            nc.tensor.matmul(ri_ps[:], lhsT=ones1[0:1, 0:128], rhs=FI[0:1, :])
            RPI = sb.tile([128, NSLOT], dt.float32, tag="SS")
            nc.scalar.copy(out=RPI[:], in_=ri_ps[:])
            GACC = sb.tile([128, 4], dt.float32, tag="GACC")
            EACC = sb.tile([128, 4], dt.float32, tag="EACC")
            SCR2 = sb.tile([128, NSLOT], dt.float32, tag="SCR2")
            for cth in range(4):
                nc.vector.tensor_scalar(
                    out=SCR2[:], in0=RPV[:], scalar1=VV[:, cth : cth + 1],
                    scalar2=0.0, op0=Alu.is_gt, op1=Alu.add,
                    accum_out=GACC[:, cth : cth + 1],
                )
                nc.vector.tensor_scalar(
                    out=SCR2[:], in0=RPI[:], scalar1=PIX[:, cth : cth + 1],
                    scalar2=None, op0=Alu.is_lt,
                )
                nc.vector.scalar_tensor_tensor(
                    out=SCR2[:], in0=RPV[:], scalar=VV[:, cth : cth + 1],
                    op0=Alu.is_equal, in1=SCR2[:], op1=Alu.mult,
                    accum_out=EACC[:, cth : cth + 1],
                )
            RANK = sb.tile([128, 4], dt.float32, tag="RANK")
            nc.vector.tensor_tensor(out=RANK[:], in0=GACC[:], in1=EACC[:], op=Alu.add)
            RKI = sb.tile([128, 4], dt.int32, tag="RKI")
            nc.vector.tensor_copy(RKI[:], RANK[:])

            # ---- scatter packed (py*1024+px) by rank ----
            PKD = sb.tile([128, 4], dt.float32, tag="PKD")
            nc.vector.scalar_tensor_tensor(
                out=PKD[:], in0=PY[:], scalar=1024.0, op0=Alu.mult, in1=PX[:],
                op1=Alu.add,
            )
            zt = sb.tile([1, NSLOT], dt.float32, tag="f3")
            nc.vector.memset(zt[:], 0.0)
            nc.sync.dma_start(kpd[:].rearrange("(o n) -> o n", o=1), zt[:])
            kview = kpd[:].rearrange("(n o) -> n o", o=1)
            for s in range(4):
                nc.gpsimd.indirect_dma_start(
                    out=kview,
                    out_offset=bass.IndirectOffsetOnAxis(
                        ap=RKI[:, s : s + 1], axis=0
                    ),
                    in_=PKD[:, s : s + 1],
                    in_offset=None,
                    bounds_check=TOPK - 1, oob_is_err=False,
                )

            # ---- readback + decode keypoints (k = 128*s + p) ----
            KP = sb.tile([128, 4], dt.float32, tag="KP")
            nc.sync.dma_start(KP[:], kpd[:].rearrange("(s p) -> p s", p=128))
            KY = sb.tile([128, 4], dt.float32, tag="KY")
            KX = sb.tile([128, 4], dt.float32, tag="KX")
            KPI = sb.tile([128, 4], dt.int32, tag="KPI")
            KYI = sb.tile([128, 4], dt.int32, tag="KYI")
            nc.vector.tensor_copy(KPI[:], KP[:])  # exact integer, any rounding
            nc.vector.tensor_scalar(
                out=KYI[:], in0=KPI[:], scalar1=10, scalar2=None,
                op0=Alu.logical_shift_right,
            )
            nc.vector.tensor_copy(KY[:], KYI[:])
            nc.vector.tensor_scalar(
                out=KYI[:], in0=KPI[:], scalar1=1023, scalar2=None,
                op0=Alu.bitwise_and,
            )
            nc.vector.tensor_copy(KX[:], KYI[:])

            # ---- patch spans (2565 elems each) + softmax refinement ----
            TB5 = sb.tile([128, 4], dt.float32, tag="TB5")
            nc.vector.scalar_tensor_tensor(
                out=TB5[:], in0=KY[:], scalar=640.0, op0=Alu.mult, in1=KX[:],
                op1=Alu.add,
            )
            nc.vector.tensor_scalar(
                out=TB5[:], in0=TB5[:], scalar1=-1282.0, scalar2=0.0,
                op0=Alu.add, op1=Alu.max,
            )
            TB5I = sb.tile([128, 4], dt.int32, tag="TB5I")
            nc.vector.tensor_copy(TB5I[:], TB5[:])
            P25 = sb.tile([128, 4, 25], dt.float32, tag="P25")
            sview = scores[:].rearrange("h w -> (h w)").rearrange("(n o) -> n o", o=1)
            for s in range(4):
                SPN = sb.tile([128, 2565], dt.float32, tag="X")
                nc.gpsimd.indirect_dma_start(
                    out=SPN[:], out_offset=None, in_=sview,
                    in_offset=bass.IndirectOffsetOnAxis(
                        ap=TB5I[:, s : s + 1], axis=0
                    ),
                )
                nc.vector.tensor_copy(
                    P25[:, s, :].rearrange("p (a b) -> p a b", a=5),
                    _sview3(SPN, 5, 640, 5),
                )
            MAXV = sb.tile([128, 4], dt.float32, tag="MAXV")
            nc.vector.tensor_reduce(out=MAXV[:], in_=P25[:], axis=Ax.X, op=Alu.max)
            XE = sb.tile([128, 4, 25], dt.float32, tag="XE")
            nc.vector.tensor_tensor(
                out=XE[:], in0=P25[:], in1=MAXV[:].to_broadcast([128, 4, 25]),
                op=Alu.subtract,
            )
            nc.scalar.activation(
                out=XE[:].rearrange("p c k -> p (c k)"),
                in_=XE[:].rearrange("p c k -> p (c k)"),
                func=ActF.Exp, scale=10.0,
            )
            SSUM = sb.tile([128, 4], dt.float32, tag="SSUM")
            nc.vector.tensor_reduce(out=SSUM[:], in_=XE[:], axis=Ax.X, op=Alu.add)
            REC = sb.tile([128, 4], dt.float32, tag="REC")
            nc.vector.tensor_scalar(
                out=REC[:], in0=SSUM[:], scalar1=1e-12, scalar2=None, op0=Alu.add
            )
            nc.vector.reciprocal(out=REC[:], in_=REC[:])
            dxr = sb.tile([128, 100], dt.float32, tag="dxr")
            dyr = sb.tile([128, 100], dt.float32, tag="dyr")
            nc.sync.dma_start(dxr[:], cst["dxr"][:])
            nc.sync.dma_start(dyr[:], cst["dyr"][:])
            T100 = sb.tile([128, 4, 25], dt.float32, tag="T100")
            XR = sb.tile([128, 4], dt.float32, tag="XR")
            YR = sb.tile([128, 4], dt.float32, tag="YR")
            for ramp, out_t in ((dxr, XR), (dyr, YR)):
                nc.vector.tensor_tensor(
                    out=T100[:], in0=XE[:],
                    in1=ramp[:].rearrange("p (c k) -> p c k", c=4), op=Alu.mult,
                )
                nc.vector.tensor_reduce(
                    out=out_t[:], in_=T100[:], axis=Ax.X, op=Alu.add
                )
                nc.vector.tensor_tensor(
                    out=out_t[:], in0=out_t[:], in1=REC[:], op=Alu.mult
                )
            D2 = sb.tile([128, 4, 25], dt.float32, tag="D2")
            nc.vector.tensor_tensor(
                out=D2[:], in0=dxr[:].rearrange("p (c k) -> p c k", c=4),
                in1=XR[:].to_broadcast([128, 4, 25]), op=Alu.subtract,
            )
            nc.vector.tensor_scalar(
                out=D2[:], in0=D2[:], scalar1=0.5, scalar2=None, op0=Alu.mult
            )
            nc.vector.tensor_tensor(out=D2[:], in0=D2[:], in1=D2[:], op=Alu.mult)
            nc.vector.tensor_tensor(
                out=T100[:], in0=dyr[:].rearrange("p (c k) -> p c k", c=4),
                in1=YR[:].to_broadcast([128, 4, 25]), op=Alu.subtract,
            )
            nc.vector.tensor_scalar(
                out=T100[:], in0=T100[:], scalar1=0.5, scalar2=None, op0=Alu.mult
            )
            nc.vector.tensor_tensor(out=T100[:], in0=T100[:], in1=T100[:], op=Alu.mult)
            nc.vector.tensor_tensor(out=D2[:], in0=D2[:], in1=T100[:], op=Alu.add)
            nc.vector.tensor_tensor(out=D2[:], in0=D2[:], in1=XE[:], op=Alu.mult)
            DISP = sb.tile([128, 4], dt.float32, tag="DISP")
            nc.vector.tensor_reduce(out=DISP[:], in_=D2[:], axis=Ax.X, op=Alu.add)
            nc.vector.tensor_tensor(out=DISP[:], in0=DISP[:], in1=REC[:], op=Alu.mult)

            # ---- normalized kxy out ----
            KXA = sb.tile([128, 4], dt.float32, tag="KXA")
            KYA = sb.tile([128, 4], dt.float32, tag="KYA")
            nc.vector.tensor_tensor(out=KXA[:], in0=KX[:], in1=XR[:], op=Alu.add)
            nc.vector.tensor_tensor(out=KYA[:], in0=KY[:], in1=YR[:], op=Alu.add)
            KXN = sb.tile([128, 4], dt.float32, tag="KXN")
            KYN = sb.tile([128, 4], dt.float32, tag="KYN")
            nc.vector.tensor_scalar(
                out=KXN[:], in0=KXA[:], scalar1=float(2.0 / (W - 1)), scalar2=-1.0,
                op0=Alu.mult, op1=Alu.add,
            )
            nc.vector.tensor_scalar(
                out=KYN[:], in0=KYA[:], scalar1=float(2.0 / (H - 1)), scalar2=-1.0,
                op0=Alu.mult, op1=Alu.add,
            )
            KXY = sb.tile([128, 4, 2], dt.float32, tag="KXY")
            nc.vector.tensor_copy(
                KXY[:, :, 0:1], KXN[:].rearrange("p (c o) -> p c o", o=1)
            )
            nc.vector.tensor_copy(
                KXY[:, :, 1:2], KYN[:].rearrange("p (c o) -> p c o", o=1)
            )
            nc.sync.dma_start(
                o_kxy[0:384, :].rearrange("(s p) d -> p s d", p=128), KXY[:, 0:3, :]
            )
            nc.sync.dma_start(
                o_kxy[384:500, :].rearrange("(s p) d -> p s d", p=116),
                KXY[0:116, 3:4, :],
            )

            # ---- grid sample coords ----
            GX = sb.tile([128, 4], dt.float32, tag="GX")
            GY = sb.tile([128, 4], dt.float32, tag="GY")
            nc.vector.tensor_scalar(
                out=GX[:], in0=KXN[:], scalar1=1.0, scalar2=float(0.5 * (W - 1)),
                op0=Alu.add, op1=Alu.mult,
            )
            nc.vector.tensor_scalar(
                out=GX[:], in0=GX[:], scalar1=0.0, scalar2=float(W - 1),
                op0=Alu.max, op1=Alu.min,
            )
            nc.vector.tensor_scalar(
                out=GY[:], in0=KYN[:], scalar1=1.0, scalar2=float(0.5 * (H - 1)),
                op0=Alu.add, op1=Alu.mult,
            )
            nc.vector.tensor_scalar(
                out=GY[:], in0=GY[:], scalar1=0.0, scalar2=float(H - 1),
                op0=Alu.max, op1=Alu.min,
            )
            X0G = sb.tile([128, 4], dt.float32, tag="X0G")
            Y0G = sb.tile([128, 4], dt.float32, tag="Y0G")
            TI = sb.tile([128, 4], dt.int32, tag="TI")
            TF = sb.tile([128, 4], dt.float32, tag="TF")
            # floor robust to cast rounding mode: y = cast(x); y -= (y > x)
            for gsrc_t, gdst, hi in ((GX, X0G, W - 2), (GY, Y0G, H - 2)):
                nc.vector.tensor_copy(TI[:], gsrc_t[:])
                nc.vector.tensor_copy(gdst[:], TI[:])
                nc.vector.tensor_tensor(
                    out=TF[:], in0=gdst[:], in1=gsrc_t[:], op=Alu.is_gt
                )
                nc.vector.tensor_tensor(
                    out=gdst[:], in0=gdst[:], in1=TF[:], op=Alu.subtract
                )
                nc.vector.tensor_scalar(
                    out=gdst[:], in0=gdst[:], scalar1=0.0, scalar2=float(hi),
                    op0=Alu.max, op1=Alu.min,
                )
            WX = sb.tile([128, 4], dt.float32, tag="WX")
            WY = sb.tile([128, 4], dt.float32, tag="WY")
            nc.vector.tensor_tensor(out=WX[:], in0=GX[:], in1=X0G[:], op=Alu.subtract)
            nc.vector.tensor_tensor(out=WY[:], in0=GY[:], in1=Y0G[:], op=Alu.subtract)
            O00 = sb.tile([128, 4], dt.float32, tag="O00")
            nc.vector.scalar_tensor_tensor(
                out=O00[:], in0=Y0G[:], scalar=640.0, op0=Alu.mult, in1=X0G[:],
                op1=Alu.add,
            )
            O00I = sb.tile([128, 4], dt.int32, tag="O00I")
            nc.vector.tensor_copy(O00I[:], O00[:])

            # ---- kptscores: 4 span gathers (642 elems) + 2x2 extract ----
            SAB = sb.tile([128, 4, 2, 2], dt.float32, tag="SAB")
            for s in range(4):
                OSP = sb.tile([128, 642], dt.float32, tag="TD")
                nc.gpsimd.indirect_dma_start(
                    out=OSP[:], out_offset=None, in_=sview,
                    in_offset=bass.IndirectOffsetOnAxis(
                        ap=O00I[:, s : s + 1], axis=0
                    ),
                )
                nc.vector.tensor_copy(SAB[:, s, :, :], _sview3(OSP, 2, 640, 2))
            TA = sb.tile([128, 4], dt.float32, tag="TA")
            TBv = sb.tile([128, 4], dt.float32, tag="TBv")
            KPT = sb.tile([128, 4], dt.float32, tag="KPT")
            nc.vector.tensor_tensor(
                out=TA[:], in0=SAB[:, :, 0, 1], in1=SAB[:, :, 0, 0], op=Alu.subtract
            )
            nc.vector.tensor_tensor(out=TA[:], in0=TA[:], in1=WX[:], op=Alu.mult)
            nc.vector.tensor_tensor(
                out=TA[:], in0=TA[:], in1=SAB[:, :, 0, 0], op=Alu.add
            )
            nc.vector.tensor_tensor(
                out=TBv[:], in0=SAB[:, :, 1, 1], in1=SAB[:, :, 1, 0], op=Alu.subtract
            )
            nc.vector.tensor_tensor(out=TBv[:], in0=TBv[:], in1=WX[:], op=Alu.mult)
            nc.vector.tensor_tensor(
                out=TBv[:], in0=TBv[:], in1=SAB[:, :, 1, 0], op=Alu.add
            )
            nc.vector.tensor_tensor(out=KPT[:], in0=TBv[:], in1=TA[:], op=Alu.subtract)
            nc.vector.tensor_tensor(out=KPT[:], in0=KPT[:], in1=WY[:], op=Alu.mult)
            nc.vector.tensor_tensor(out=KPT[:], in0=KPT[:], in1=TA[:], op=Alu.add)
            nc.sync.dma_start(
                o_kpt[0:384].rearrange("(s p) -> p s", p=128), KPT[:, 0:3]
            )
            nc.sync.dma_start(
                o_kpt[384:500].rearrange("(s p) -> p s", p=116), KPT[0:116, 3:4]
            )
            nc.sync.dma_start(
                o_disp[0:384].rearrange("(s p) -> p s", p=128), DISP[:, 0:3]
            )
            nc.sync.dma_start(
                o_disp[384:500].rearrange("(s p) -> p s", p=116), DISP[0:116, 3:4]
            )

            # ---- descriptor sampling from descT (keypoint-major) ----
            O00T = sb.tile([128, 4], dt.int32, tag="O00T")
            O10T = sb.tile([128, 4], dt.int32, tag="O10T")
            nc.vector.tensor_scalar(
                out=O10T[:], in0=O00I[:], scalar1=640, scalar2=None, op0=Alu.add
            )
            nc.vector.tensor_scalar(
                out=O00T[:], in0=O00I[:], scalar1=6, scalar2=None,
                op0=Alu.logical_shift_left,
            )
            nc.vector.tensor_scalar(
                out=O10T[:], in0=O10T[:], scalar1=6, scalar2=None,
                op0=Alu.logical_shift_left,
            )
            tflat = descT[:].rearrange("(n o) -> n o", o=1)
            DGA = sb.tile([128, 4, 128], dt.bfloat16, tag="DGA")
            DGB = sb.tile([128, 4, 128], dt.bfloat16, tag="DGB")
            for s in range(4):
                nc.gpsimd.indirect_dma_start(
                    out=DGA[:, s, :], out_offset=None, in_=tflat,
                    in_offset=bass.IndirectOffsetOnAxis(
                        ap=O00T[:, s : s + 1], axis=0
                    ),
                )
                nc.gpsimd.indirect_dma_start(
                    out=DGB[:, s, :], out_offset=None, in_=tflat,
                    in_offset=bass.IndirectOffsetOnAxis(
                        ap=O10T[:, s : s + 1], axis=0
                    ),
                )
            DA = sb.tile([128, 4, 64], dt.float32, tag="DA")
            DB = sb.tile([128, 4, 64], dt.float32, tag="DB")
            DSm = sb.tile([128, 4, 64], dt.float32, tag="DSm")
            nc.vector.tensor_tensor(
                out=DA[:], in0=DGA[:, :, 64:128], in1=DGA[:, :, 0:64], op=Alu.subtract
            )
            nc.vector.tensor_tensor(
                out=DA[:], in0=DA[:], in1=WX[:].to_broadcast([128, 4, 64]), op=Alu.mult
            )
            nc.vector.tensor_tensor(
                out=DA[:], in0=DA[:], in1=DGA[:, :, 0:64], op=Alu.add
            )
            nc.vector.tensor_tensor(
                out=DB[:], in0=DGB[:, :, 64:128], in1=DGB[:, :, 0:64], op=Alu.subtract
            )
            nc.vector.tensor_tensor(
                out=DB[:], in0=DB[:], in1=WX[:].to_broadcast([128, 4, 64]), op=Alu.mult
            )
            nc.vector.tensor_tensor(
                out=DB[:], in0=DB[:], in1=DGB[:, :, 0:64], op=Alu.add
            )
            nc.vector.tensor_tensor(out=DSm[:], in0=DB[:], in1=DA[:], op=Alu.subtract)
            nc.vector.tensor_tensor(
                out=DSm[:], in0=DSm[:], in1=WY[:].to_broadcast([128, 4, 64]),
                op=Alu.mult,
            )
            nc.vector.tensor_tensor(out=DSm[:], in0=DSm[:], in1=DA[:], op=Alu.add)
            DSQ = sb.tile([128, 4, 64], dt.float32, tag="DSQ")
            nc.vector.tensor_tensor(out=DSQ[:], in0=DSm[:], in1=DSm[:], op=Alu.mult)
            NRM = sb.tile([128, 4], dt.float32, tag="NRM")
            nc.vector.tensor_reduce(out=NRM[:], in_=DSQ[:], axis=Ax.X, op=Alu.add)
            nc.scalar.activation(out=NRM[:], in_=NRM[:], func=ActF.Sqrt)
            nc.vector.tensor_scalar(
                out=NRM[:], in0=NRM[:], scalar1=1e-12, scalar2=None, op0=Alu.max
            )
            nc.vector.reciprocal(out=NRM[:], in_=NRM[:])
            nc.vector.tensor_tensor(
                out=DSm[:], in0=DSm[:], in1=NRM[:].to_broadcast([128, 4, 64]),
                op=Alu.mult,
            )
            nc.sync.dma_start(
                o_desc[0:384, :].rearrange("(s p) d -> p s d", p=128), DSm[:, 0:3, :]
            )
            nc.sync.dma_start(
                o_desc[384:500, :].rearrange("(s p) d -> p s d", p=116),
                DSm[0:116, 3:4, :],
            )
    nc.finalize()
    return nc


_NC_CACHE = None


def _get_nc():
    global _NC_CACHE
    if _NC_CACHE is None:
        _NC_CACHE = build_nc()
    return _NC_CACHE


def kernel(scores_map, descriptor_map, sub_pixel=1, _want_time=False):
    scores_map = np.asarray(scores_map, dtype=np.float32)
    descriptor_map = np.asarray(descriptor_map, dtype=np.float32)
    B = scores_map.shape[0]
    nc = _get_nc()
    in_maps = []
    for b in range(B):
        m = {
            "scores": np.ascontiguousarray(scores_map[b, 0]),
            "descf": np.ascontiguousarray(descriptor_map[b].reshape(-1)),
        }
        for k, v in CONSTS.items():
            m[k] = v
        in_maps.append(m)
    try:
        res = run_bass_kernel_spmd(nc, in_maps, list(range(B)), trace=_want_time)
    except ModuleNotFoundError:
        res = run_bass_kernel_spmd(nc, in_maps, list(range(B)), trace=False)
    outs = res.results
    kxy = np.stack([o["out_kxy"] for o in outs])
    desc = np.stack([o["out_desc"] for o in outs])
    kpt = np.stack([o["out_kpt"] for o in outs])
    disp = np.stack([o["out_disp"] for o in outs])
    if _want_time:
        t = res.exec_time_ns
        if t is None:
            # no NTFF profiling hook in this environment: report the
            # cost-model timeline estimate for a single core instead
            from concourse.timeline_sim import TimelineSim

            t = int(TimelineSim(nc).simulate())
        return (kxy, desc, kpt, disp), t
    return kxy, desc, kpt, disp


# revision 41
# speedup vs baseline: 3.9541x; 3.9541x over previous
"""DKD keypoint-detection kernel for Trainium2 (Bass/Tile), batch-parallel on 8 cores.

Per core (one image [1,480,640] scores + [64,480,640] desc):
  NMS (5 maxpools, ping-pong layouts via PE transpose) -> 3x3 block-reduce with
  argmax carry -> qkey (value mantissa | 13-bit position code) -> per-partition
  top-32 (max8/match_replace) -> kth_largest prune -> sparse_gather compaction
  -> exact lex rank (value desc, pixel idx asc) by counting -> rank scatter ->
  span-gather patches + softmax refinement -> bilinear grid-sample of scores
  and descriptors (bf16 transposed desc copy built on device, overlapped with
  NMS) -> L2 normalize.  Outputs kxy [500,2], desc [500,64], kpt [500],
  disp [500].

HW notes baked in: indirect DMA supports ONE index per partition-row only;
DVE int arithmetic runs through the fp32 ALU (exact only below 2^24);
f32->i32 cast rounds to nearest (not trunc).
"""
import sys

sys.path.insert(0, "/opt/trn_rl_repo")
import numpy as np

import concourse.bass as bass
import concourse.bacc as bacc
import concourse.mybir as mybir
import concourse.tile as tile
from concourse.bass_utils import run_bass_kernel_spmd

dt = mybir.dt
Alu = mybir.AluOpType
ActF = mybir.ActivationFunctionType
Ax = mybir.AxisListType

H, W = 480, 640
HW = H * W
HP = 120          # rows per chunk (4 chunks) in A layout
NBW = 213         # 3-pixel block columns (col 639 always zero)
NJ = 426          # candidate slots per partition = 2 (hb) * 213 (wb)
NKEEP = 510       # prune target: t = NKEEP-th largest qkey
NSLOT = 512
TOPK = 500
C1 = float(0.5 - 2.0 ** -15)  # round-to-nearest -> floor helper
DCH = 150         # desc transpose chunks of 2048 pixels
NMS_ITERS = 2     # bisect knob
STOP_EARLY = 0    # 1=stop after nms, 2=stop after rank scatter


def _consts():
    c = {}
    c["ident"] = np.eye(128, dtype=np.float32)
    c["identb"] = np.eye(128, dtype=np.float32)
    c["ones1"] = np.ones((1, 128), np.float32)
    j = np.arange(NJ)
    hb, wb = j // NBW, j % NBW
    c["jor_e"] = np.tile((511 - (hb * 256 + wb)).astype(np.int32), (128, 1))
    i3 = np.tile(np.repeat(np.arange(3, dtype=np.float32), 3), 4)   # col offset
    r3 = np.tile(np.tile(np.arange(3, dtype=np.float32), 3), 4)    # row offset
    c["cc36"] = np.tile(i3, (128, 1))
    c["rr36"] = np.tile(r3, (128, 1))
    c["p3"] = (3.0 * np.arange(128, dtype=np.float32))[:, None]
    col = np.arange(128)[:, None] + 128 * np.arange(5)[None, :]
    cm = ((col >= 3) & (col <= 637)).astype(np.float32)
    c["colmask"] = cm.reshape(128, 5, 1).copy()
    dx = np.tile(np.tile(np.arange(5, dtype=np.float32) - 2, 5), 4)
    dy = np.tile(np.repeat(np.arange(5, dtype=np.float32) - 2, 5), 4)
    c["dxr"] = np.tile(dx, (128, 1))
    c["dyr"] = np.tile(dy, (128, 1))
    return c


CONSTS = _consts()
_CONST_DT = {"jor_e": dt.int32}


def _mp_h(nc, sb, src, dst):
    """5-tap max along cols (2..641) of [120,4,648] A-layout tiles."""
    h4 = sb.tile([HP, 4, 648], dt.float32, tag="mp_h4")
    nc.vector.tensor_tensor(
        out=dst[:, :, 0:646], in0=src[:, :, 0:646], in1=src[:, :, 1:647], op=Alu.max
    )
    nc.vector.tensor_tensor(
        out=h4[:, :, 0:644], in0=dst[:, :, 0:644], in1=dst[:, :, 2:646], op=Alu.max
    )
    nc.vector.tensor_tensor(
        out=dst[:, :, 2:642], in0=h4[:, :, 0:640], in1=src[:, :, 4:644], op=Alu.max
    )


def _mp_v(nc, sb, src, dst):
    """5-tap max along h (2..481) of [128,5,484] B-layout tiles."""
    v4 = sb.tile([128, 5, 484], dt.float32, tag="mp_v4")
    nc.vector.tensor_tensor(
        out=dst[:, :, 0:483], in0=src[:, :, 0:483], in1=src[:, :, 1:484], op=Alu.max
    )
    nc.vector.tensor_tensor(
        out=v4[:, :, 0:481], in0=dst[:, :, 0:481], in1=dst[:, :, 2:483], op=Alu.max
    )
    nc.vector.tensor_tensor(
        out=dst[:, :, 2:482], in0=v4[:, :, 0:480], in1=src[:, :, 4:484], op=Alu.max
    )


def _t_a2b(nc, ps, ident, src, dst):
    """Transpose A[120,4,648](cols 2..641) -> B[128,5,484](h 2..481)."""
    tp = ps.tile([128, 5, 512], dt.float32, tag="ps_big")
    for c in range(4):
        for w in range(5):
            nc.tensor.transpose(
                out=tp[:, w, c * HP : (c + 1) * HP],
                in_=src[0:HP, c, 2 + 128 * w : 2 + 128 * (w + 1)],
                identity=ident[0:HP, 0:HP],
            )
    nc.scalar.copy(out=dst[:, :, 2:482], in_=tp[:, :, 0:480])


def _t_b2a(nc, ps, ident, src, dst):
    """Transpose B[128,5,484](h 2..481) -> A[120,4,648](cols 2..641)."""
    tp = ps.tile([HP, 4, 5, 128], dt.float32, tag="ps_big")
    for c in range(4):
        for w in range(5):
            nc.tensor.transpose(
                out=tp[:, c, w, :],
                in_=src[0:128, w, 2 + HP * c : 2 + HP * (c + 1)],
                identity=ident[0:128, 0:128],
            )
    nc.scalar.copy(
        out=dst[:, :, 2:642], in_=tp[:].rearrange("p c w q -> p c (w q)")
    )


def _sview3(t, n_mid, stride_mid, n_in):
    """[128, F] tile -> AP [128, n_mid, n_in] with mid-stride over free dim."""
    base = t[:]
    return bass.AP(base.tensor, base.offset, [list(base.ap[0]), [stride_mid, n_mid], [1, n_in]])


def build_nc(dbg=False):
    nc = bacc.Bacc(None, target_bir_lowering=False)
    scores = nc.dram_tensor("scores", [H, W], dt.float32, kind="ExternalInput")
    descf = nc.dram_tensor("descf", [64 * HW], dt.float32, kind="ExternalInput")
    cst = {}
    for k, v in CONSTS.items():
        cst[k] = nc.dram_tensor(
            k, list(v.shape), _CONST_DT.get(k, dt.float32), kind="ExternalInput"
        )

    o_kxy = nc.dram_tensor("out_kxy", [TOPK, 2], dt.float32, kind="ExternalOutput")
    o_desc = nc.dram_tensor("out_desc", [TOPK, 64], dt.float32, kind="ExternalOutput")
    o_kpt = nc.dram_tensor("out_kpt", [TOPK], dt.float32, kind="ExternalOutput")
    o_disp = nc.dram_tensor("out_disp", [TOPK], dt.float32, kind="ExternalOutput")

    kw = dict(kind="ExternalOutput") if dbg else {}
    nmsd = nc.dram_tensor("nmsd", [W * H], dt.float32, **kw)  # col-major nms
    vseld = nc.dram_tensor("vseld", [128 * 32], dt.float32, **kw)
    vseld2 = nc.dram_tensor("vseld2", [128 * 32], dt.float32, **kw)
    sgd = nc.dram_tensor("sgd", [NSLOT], dt.float32, **kw)
    sgd2 = nc.dram_tensor("sgd2", [NSLOT], dt.float32, **kw)
    vvd = nc.dram_tensor("vvd", [NSLOT], dt.float32, **kw)
    pixd = nc.dram_tensor("pixd", [NSLOT], dt.float32, **kw)
    kpd = nc.dram_tensor("kpd", [NSLOT], dt.float32, **kw)
    descT = nc.dram_tensor("descT", [HW * 64], dt.bfloat16)

    with tile.TileContext(nc) as tc:
        with tc.tile_pool(name="sb", bufs=1) as sb, tc.tile_pool(
            name="ps", bufs=1, space="PSUM"
        ) as ps, tc.tile_pool(name="dsb", bufs=3) as dsb, tc.tile_pool(
            name="dps", bufs=2, space="PSUM"
        ) as dps:
            ident = sb.tile([128, 128], dt.float32, tag="ident")
            identb = sb.tile([128, 128], dt.bfloat16, tag="identb")
            ones1 = sb.tile([1, 128], dt.float32, tag="ones1")
            nc.sync.dma_start(ident[:], cst["ident"][:])
            nc.gpsimd.dma_start(identb[:], cst["identb"][:])  # f32->bf16 cast DMA
            nc.sync.dma_start(ones1[:], cst["ones1"][:])

            # ===== descriptor transpose to [HW, 64] bf16 (overlappable) =====
            dview = descf[:].rearrange("(c x) -> c x", c=64)
            tview = descT[:].rearrange("(x c) -> x c", c=64)
            for ci in range(DCH):
                x0 = 2048 * ci
                stg = dsb.tile([64, 2048], dt.bfloat16, tag="dstg")
                nc.gpsimd.dma_start(stg[:], dview[:, x0 : x0 + 2048])
                dout = dsb.tile([128, 16, 64], dt.bfloat16, tag="dout")
                for half in range(2):
                    pt = dps.tile([128, 8, 64], dt.bfloat16, tag="ps_d")
                    for b in range(8):
                        bb = half * 8 + b
                        nc.tensor.transpose(
                            out=pt[:, b, :],
                            in_=stg[0:64, 128 * bb : 128 * (bb + 1)],
                            identity=identb[0:64, 0:64],
                        )
                    if (ci + half) % 2 == 0:
                        nc.vector.tensor_copy(
                            dout[:, half * 8 : half * 8 + 8, :], pt[:]
                        )
                    else:
                        nc.scalar.copy(
                            out=dout[:, half * 8 : half * 8 + 8, :], in_=pt[:]
                        )
                nc.sync.dma_start(
                    tview[x0 : x0 + 2048, :].rearrange("(b p) c -> p b c", p=128),
                    dout[:],
                )

            # ========================= NMS ==================================
            def A(tag):
                t = sb.tile([HP, 4, 648], dt.float32, tag=tag, name=tag)
                return t

            def B(tag):
                t = sb.tile([128, 5, 484], dt.float32, tag=tag, name=tag)
                return t

            X = A("X")
            nc.vector.memset(X[:, :, 0:2], 0.0)
            nc.vector.memset(X[:, :, 642:648], 0.0)
            nc.sync.dma_start(
                X[:, :, 2:642], scores[:].rearrange("(c p) w -> p c w", p=HP)
            )
            sT = B("sT")
            _t_a2b(nc, ps, ident, X, sT)

            HH = A("HH")
            _mp_h(nc, sb, X, HH)
            HB = B("HB")
            nc.vector.memset(HB[:, :, 0:2], 0.0)
            nc.vector.memset(HB[:, :, 482:484], 0.0)
            _t_a2b(nc, ps, ident, HH, HB)
            MP = B("MP")
            _mp_v(nc, sb, HB, MP)

            M1 = B("M1")
            nc.vector.memset(M1[:, :, 0:2], 0.0)
            nc.vector.memset(M1[:, :, 482:484], 0.0)
            nc.vector.tensor_tensor(
                out=M1[:, :, 2:482], in0=sT[:, :, 2:482], in1=MP[:, :, 2:482],
                op=Alu.is_equal,
            )

            SS = A("SS")
            nc.vector.memset(SS[:, :, 0:2], 0.0)
            nc.vector.memset(SS[:, :, 642:648], 0.0)
            SSB = B("SSB")
            D_A = A("D_A")
            D_B = B("D_B")
            NEW = B("NEW")

            for it in range(NMS_ITERS):
                VD = sb.tile([128, 5, 484], dt.float32, tag="HB")
                _mp_v(nc, sb, M1, VD)
                TD = sb.tile([HP, 4, 648], dt.float32, tag="TD")
                nc.vector.memset(TD[:, :, 0:2], 0.0)
                nc.vector.memset(TD[:, :, 642:648], 0.0)
                _t_b2a(nc, ps, ident, VD, TD)
                _mp_h(nc, sb, TD, D_A)
                _t_a2b(nc, ps, ident, D_A, D_B)
                nc.vector.tensor_scalar(
                    out=D_A[:, :, 2:642], in0=D_A[:, :, 2:642],
                    scalar1=0.0, scalar2=None, op0=Alu.is_gt,
                )
                nc.vector.tensor_scalar(
                    out=D_B[:, :, 2:482], in0=D_B[:, :, 2:482],
                    scalar1=0.0, scalar2=None, op0=Alu.is_gt,
                )
                nc.vector.tensor_scalar(
                    out=SS[:, :, 2:642], in0=D_A[:, :, 2:642],
                    scalar1=-1.0, scalar2=1.0, op0=Alu.mult, op1=Alu.add,
                )
                nc.vector.tensor_tensor(
                    out=SS[:, :, 2:642], in0=SS[:, :, 2:642], in1=X[:, :, 2:642],
                    op=Alu.mult,
                )
                nc.vector.tensor_scalar(
                    out=SSB[:, :, 2:482], in0=D_B[:, :, 2:482],
                    scalar1=-1.0, scalar2=1.0, op0=Alu.mult, op1=Alu.add,
                )
                nc.vector.tensor_tensor(
                    out=SSB[:, :, 2:482], in0=SSB[:, :, 2:482], in1=sT[:, :, 2:482],
                    op=Alu.mult,
                )
                HH2 = sb.tile([HP, 4, 648], dt.float32, tag="HH")
                _mp_h(nc, sb, SS, HH2)
                HB2 = sb.tile([128, 5, 484], dt.float32, tag="HB")
                nc.vector.memset(HB2[:, :, 0:2], 0.0)
                nc.vector.memset(HB2[:, :, 482:484], 0.0)
                _t_a2b(nc, ps, ident, HH2, HB2)
                _mp_v(nc, sb, HB2, MP)
                nc.vector.tensor_tensor(
                    out=NEW[:, :, 2:482], in0=SSB[:, :, 2:482], in1=MP[:, :, 2:482],
                    op=Alu.is_equal,
                )
                nc.vector.scalar_tensor_tensor(
                    out=NEW[:, :, 2:482], in0=D_B[:, :, 2:482], scalar=-1.0,
                    op0=Alu.mult, in1=NEW[:, :, 2:482], op1=Alu.add,
                )
                nc.vector.tensor_scalar(
                    out=NEW[:, :, 2:482], in0=NEW[:, :, 2:482],
                    scalar1=0.0, scalar2=None, op0=Alu.is_gt,
                )
                nc.vector.tensor_tensor(
                    out=M1[:, :, 2:482], in0=M1[:, :, 2:482], in1=NEW[:, :, 2:482],
                    op=Alu.max,
                )

            NMS = sb.tile([128, 5, 484], dt.float32, tag="SSB")
            nc.vector.tensor_tensor(
                out=NMS[:, :, 2:482], in0=sT[:, :, 2:482], in1=M1[:, :, 2:482],
                op=Alu.mult,
            )
            cmk = sb.tile([128, 5, 1], dt.float32, tag="cmk")
            nc.sync.dma_start(cmk[:], cst["colmask"][:])
            nc.vector.tensor_tensor(
                out=NMS[:, :, 2:482], in0=NMS[:, :, 2:482],
                in1=cmk[:].to_broadcast([128, 5, 480]), op=Alu.mult,
            )
            nc.vector.memset(NMS[:, :, 2:5], 0.0)
            nc.vector.memset(NMS[:, :, 480:482], 0.0)
            nc.sync.dma_start(
                nmsd[:].rearrange("(w p h) -> p w h", p=128, h=H), NMS[:, :, 2:482]
            )

            if STOP_EARLY == 1:
                zk = sb.tile([128, 4, 2], dt.float32, tag="zk")
                nc.vector.memset(zk[:], 0.0)
                nc.sync.dma_start(
                    o_kxy[0:384, :].rearrange("(s p) d -> p s d", p=128), zk[:, 0:3, :]
                )
                nc.sync.dma_start(
                    o_kxy[384:500, :].rearrange("(s p) d -> p s d", p=116),
                    zk[0:116, 3:4, :],
                )
                zd = sb.tile([128, 4, 64], dt.float32, tag="zd")
                nc.vector.memset(zd[:], 0.0)
                nc.sync.dma_start(
                    o_desc[0:384, :].rearrange("(s p) d -> p s d", p=128),
                    zd[:, 0:3, :],
                )
                nc.sync.dma_start(
                    o_desc[384:500, :].rearrange("(s p) d -> p s d", p=116),
                    zd[0:116, 3:4, :],
                )
                nc.sync.dma_start(
                    o_kpt[0:384].rearrange("(s p) -> p s", p=128), zk[:, 0:3, 0]
                )
                nc.sync.dma_start(
                    o_kpt[384:500].rearrange("(s p) -> p s", p=116), zk[0:116, 3:4, 0]
                )
                nc.sync.dma_start(
                    o_disp[0:384].rearrange("(s p) -> p s", p=128), zk[:, 0:3, 1]
                )
                nc.sync.dma_start(
                    o_disp[384:500].rearrange("(s p) -> p s", p=116), zk[0:116, 3:4, 1]
                )
                nc.finalize()
                return nc
            # ============ 3x3 block reduce with argmax carry ================
            a0 = NMS[:, :, 2:482:3]
            a1 = NMS[:, :, 3:482:3]
            a2 = NMS[:, :, 4:482:3]
            HBK = sb.tile([128, 5, 160], dt.float32, tag="HBK")
            nc.vector.tensor_tensor(out=HBK[:], in0=a0, in1=a1, op=Alu.max)
            nc.vector.tensor_tensor(out=HBK[:], in0=HBK[:], in1=a2, op=Alu.max)
            CS = sb.tile([128, 2, 640], dt.float32, tag="CS")
            ctp = ps.tile([128, 2, 5, 128], dt.float32, tag="ps_big")
            for w in range(5):
                nc.tensor.transpose(
                    out=ctp[:, 0, w, :], in_=HBK[0:128, w, 0:128],
                    identity=ident[0:128, 0:128],
                )
                nc.tensor.transpose(
                    out=ctp[0:32, 1, w, :], in_=HBK[0:128, w, 128:160],
                    identity=ident[0:128, 0:128],
                )
            nc.vector.memset(CS[32:64, 1, :], 0.0)
            nc.vector.memset(CS[64:128, 1, :], 0.0)
            nc.scalar.copy(
                out=CS[:, 0, :], in_=ctp[:, 0, :, :].rearrange("p w q -> p (w q)")
            )
            nc.scalar.copy(
                out=CS[0:32, 1, :],
                in_=ctp[0:32, 1, :, :].rearrange("p w q -> p (w q)"),
            )
            C = sb.tile([128, 2, NBW], dt.float32, tag="C")
            nc.vector.tensor_tensor(
                out=C[:], in0=CS[:, :, 0:639:3], in1=CS[:, :, 1:639:3], op=Alu.max
            )
            nc.vector.tensor_tensor(
                out=C[:], in0=C[:], in1=CS[:, :, 2:639:3], op=Alu.max
            )
            jor = sb.tile([128, NJ], dt.int32, tag="jor")
            nc.sync.dma_start(jor[:], cst["jor_e"][:])
            QK = sb.tile([128, NJ], dt.int32, tag="QK")
            QKA = sb.tile([128, NJ], dt.int32, tag="QKA")
            nc.vector.tensor_scalar(
                out=QKA[:],
                in0=C[:].rearrange("p a b -> p (a b)").bitcast(dt.int32),
                scalar1=~0x1FF, scalar2=None, op0=Alu.bitwise_and,
            )
            nc.vector.tensor_tensor(
                out=QK[:], in0=QKA[:], in1=jor[:], op=Alu.bitwise_or
            )
            QKf = QK[:].bitcast(dt.float32)

            # ---- per-partition top-32 ----
            V = sb.tile([128, 32], dt.float32, tag="V")
            for r in range(4):
                nc.vector.max(out=V[:, r * 8 : (r + 1) * 8], in_=QKf)
                nc.vector.match_replace(
                    out=QKf, in_to_replace=V[:, r * 8 : (r + 1) * 8],
                    in_values=QKf, imm_value=-1.0,
                )

            # ---- kth_largest -> threshold broadcast ----
            ko = sb.tile([1, 2], dt.float32, tag="ko")
            q = 1.0 - (NKEEP - 1.5) / (128 * 32 - 1)
            nc.gpsimd.kth_largest(
                out_ap=ko[:], in_ap=V[:], n_per_lane=32, k=NKEEP - 1, quantile=q
            )
            tb_ps = ps.tile([128, 1], dt.float32, tag="ps_rep")
            nc.tensor.matmul(tb_ps[:], lhsT=ones1[0:1, 0:128], rhs=ko[0:1, 1:2])
            tb = sb.tile([128, 1], dt.float32, tag="tb")
            nc.vector.tensor_copy(tb[:], tb_ps[:])

            # ---- compaction via sparse_gather ----
            MM = sb.tile([128, 32], dt.int32, tag="MM")
            VS = sb.tile([128, 32], dt.float32, tag="VS")
            nc.vector.tensor_scalar(
                out=MM[:], in0=V[:], scalar1=tb[:, 0:1], scalar2=None, op0=Alu.is_ge
            )
            p3 = sb.tile([128, 1], dt.float32, tag="p3")
            nc.sync.dma_start(p3[:], cst["p3"][:])
            nc.vector.memset(VS[:], -1.0)
            nc.vector.copy_predicated(out=VS[:], mask=MM[:], data=V[:])
            nc.sync.dma_start(vseld[:].rearrange("(p i) -> p i", p=128), VS[:])
            VS2 = sb.tile([128, 32], dt.float32, tag="VS2")
            nc.vector.memset(VS2[:], -1.0)
            nc.vector.copy_predicated(
                out=VS2[:], mask=MM[:], data=p3[:].to_broadcast([128, 32])
            )
            nc.sync.dma_start(vseld2[:].rearrange("(p i) -> p i", p=128), VS2[:])
            W16 = sb.tile([16, 256], dt.float32, tag="W16")
            SGO = sb.tile([16, 32], dt.float32, tag="SGO")
            nf = sb.tile([1, 1], dt.uint32, tag="nf")
            CQ = sb.tile([128, 4], dt.float32, tag="CQ")
            CP3 = sb.tile([128, 4], dt.float32, tag="CP3")
            for src_d, dst_d, dst_t in ((vseld, sgd, CQ), (vseld2, sgd2, CP3)):
                nc.sync.dma_start(W16[:], src_d[:].rearrange("(f q) -> q f", q=16))
                nc.vector.memset(SGO[:], -1.0)
                nc.gpsimd.sparse_gather(out=SGO[:], in_=W16[:], num_found=nf[:])
                nc.sync.dma_start(dst_d[:].rearrange("(f q) -> q f", q=16), SGO[:])
                nc.sync.dma_start(
                    dst_t[:], dst_d[:].rearrange("(s p) -> p s", p=128)
                )

            # ---- decode compacted qkeys ----
            KM = sb.tile([128, 4], dt.float32, tag="KM")
            nc.vector.tensor_scalar(
                out=KM[:], in0=CQ[:], scalar1=0.0, scalar2=None, op0=Alu.is_ge
            )
            E9 = sb.tile([128, 4], dt.int32, tag="E9")
            nc.vector.tensor_scalar(
                out=E9[:], in0=CQ[:].bitcast(dt.int32), scalar1=0x1FF,
                scalar2=None, op0=Alu.bitwise_and,
            )
            nc.vector.tensor_scalar(
                out=E9[:], in0=E9[:], scalar1=-1, scalar2=511,
                op0=Alu.mult, op1=Alu.add,
            )
            HBI = sb.tile([128, 4], dt.int32, tag="HBI")
            WBI = sb.tile([128, 4], dt.int32, tag="WBI")
            nc.vector.tensor_scalar(
                out=HBI[:], in0=E9[:], scalar1=8, scalar2=None,
                op0=Alu.logical_shift_right,
            )
            nc.vector.tensor_scalar(
                out=WBI[:], in0=E9[:], scalar1=255, scalar2=None, op0=Alu.bitwise_and
            )
            HBF = sb.tile([128, 4], dt.float32, tag="HBF")
            WBF = sb.tile([128, 4], dt.float32, tag="WBF")
            nc.vector.tensor_copy(HBF[:], HBI[:])
            nc.vector.tensor_copy(WBF[:], WBI[:])
            PY0 = sb.tile([128, 4], dt.float32, tag="PY0")
            PX0 = sb.tile([128, 4], dt.float32, tag="PX0")
            nc.vector.scalar_tensor_tensor(
                out=PY0[:], in0=HBF[:], scalar=384.0, op0=Alu.mult,
                in1=CP3[:], op1=Alu.add,
            )
            nc.vector.tensor_scalar(
                out=PX0[:], in0=WBF[:], scalar1=3.0, scalar2=None, op0=Alu.mult
            )
            # block span (col-major nms): 2 cols + 3 = 963 elems from px0*480+py0
            SPI = sb.tile([128, 4], dt.float32, tag="SPI")
            nc.vector.scalar_tensor_tensor(
                out=SPI[:], in0=PX0[:], scalar=480.0, op0=Alu.mult, in1=PY0[:],
                op1=Alu.add,
            )
            nc.vector.tensor_scalar(
                out=SPI[:], in0=SPI[:], scalar1=0.0, scalar2=float(HW - 963),
                op0=Alu.max, op1=Alu.min,
            )
            SPII = sb.tile([128, 4], dt.int32, tag="SPII")
            nc.vector.tensor_copy(SPII[:], SPI[:])
            B9 = sb.tile([128, 4, 9], dt.float32, tag="B9")
            nview = nmsd[:].rearrange("(n o) -> n o", o=1)
            CIDX = sb.tile([128, 4], dt.int32, tag="CIDX")
            for i in range(3):
                nc.vector.tensor_scalar(
                    out=CIDX[:], in0=SPII[:], scalar1=480 * i, scalar2=None,
                    op0=Alu.add,
                )
                for s in range(4):
                    nc.gpsimd.indirect_dma_start(
                        out=B9[:, s, 3 * i : 3 * i + 3], out_offset=None, in_=nview,
                        in_offset=bass.IndirectOffsetOnAxis(
                            ap=CIDX[:, s : s + 1], axis=0
                        ),
                    )
            VV = sb.tile([128, 4], dt.float32, tag="VV")
            nc.vector.tensor_reduce(out=VV[:], in_=B9[:], axis=Ax.X, op=Alu.max)
            nc.vector.tensor_tensor(out=VV[:], in0=VV[:], in1=KM[:], op=Alu.mult)
            EQ = sb.tile([128, 36], dt.float32, tag="EQ")
            nc.vector.tensor_tensor(
                out=EQ[:].rearrange("p (c k) -> p c k", c=4),
                in0=B9[:], in1=VV[:].to_broadcast([128, 4, 9]), op=Alu.is_equal,
            )
            rr36 = sb.tile([128, 36], dt.float32, tag="rr36")
            cc36 = sb.tile([128, 36], dt.float32, tag="cc36")
            nc.sync.dma_start(rr36[:], cst["rr36"][:])
            nc.sync.dma_start(cc36[:], cst["cc36"][:])
            T36 = sb.tile([128, 36], dt.float32, tag="T36")
            PY = sb.tile([128, 4], dt.float32, tag="PY")
            PX = sb.tile([128, 4], dt.float32, tag="PX")
            nc.vector.tensor_tensor(out=T36[:], in0=EQ[:], in1=rr36[:], op=Alu.mult)
            nc.vector.tensor_reduce(
                out=PY[:], in_=T36[:].rearrange("p (c k) -> p c k", c=4),
                axis=Ax.X, op=Alu.add,
            )
            nc.vector.tensor_tensor(out=PY[:], in0=PY[:], in1=PY0[:], op=Alu.add)
            nc.vector.tensor_tensor(out=T36[:], in0=EQ[:], in1=cc36[:], op=Alu.mult)
            nc.vector.tensor_reduce(
                out=PX[:], in_=T36[:].rearrange("p (c k) -> p c k", c=4),
                axis=Ax.X, op=Alu.add,
            )
            nc.vector.tensor_tensor(out=PX[:], in0=PX[:], in1=PX0[:], op=Alu.add)
            PIX = sb.tile([128, 4], dt.float32, tag="PIX")
            nc.vector.scalar_tensor_tensor(
                out=PIX[:], in0=PY[:], scalar=640.0, op0=Alu.mult, in1=PX[:],
                op1=Alu.add,
            )

            # ---- replicate (VV, PIX); exact lex rank by counting ----
            nc.sync.dma_start(vvd[:].rearrange("(s p) -> p s", p=128), VV[:])
            nc.sync.dma_start(pixd[:].rearrange("(s p) -> p s", p=128), PIX[:])
            FV = sb.tile([1, NSLOT], dt.float32, tag="f1")
            FI = sb.tile([1, NSLOT], dt.float32, tag="f2")
            nc.sync.dma_start(FV[:], vvd[:].rearrange("(o n) -> o n", o=1))
            nc.sync.dma_start(FI[:], pixd[:].rearrange("(o n) -> o n", o=1))
            rv_ps = ps.tile([128, NSLOT], dt.float32, tag="ps_rep")
            nc.tensor.matmul(rv_ps[:], lhsT=ones1[0:1, 0:128], rhs=FV[0:1, :])
            RPV = sb.tile([128, NSLOT], dt.float32, tag="D_A")
            nc.scalar.copy(out=RPV[:], in_=rv_ps[:])
            ri_ps = ps.tile([128, NSLOT], dt.float32, tag="ps_rep")
            nc.tensor.matmul(ri_ps[:], lhsT=ones1[0:1, 0:128], rhs=FI[0:1, :])
            RPI = sb.tile([128, NSLOT], dt.float32, tag="SS")
            nc.scalar.copy(out=RPI[:], in_=ri_ps[:])
            GACC = sb.tile([128, 4], dt.float32, tag="GACC")
            EACC = sb.tile([128, 4], dt.float32, tag="EACC")
            SCR2 = sb.tile([128, NSLOT], dt.float32, tag="SCR2")
            for cth in range(4):
                nc.vector.tensor_scalar(
                    out=SCR2[:], in0=RPV[:], scalar1=VV[:, cth : cth + 1],
                    scalar2=0.0, op0=Alu.is_gt, op1=Alu.add,
                    accum_out=GACC[:, cth : cth + 1],
                )
                nc.vector.tensor_scalar(
                    out=SCR2[:], in0=RPI[:], scalar1=PIX[:, cth : cth + 1],
                    scalar2=None, op0=Alu.is_lt,
                )
                nc.vector.scalar_tensor_tensor(
                    out=SCR2[:], in0=RPV[:], scalar=VV[:, cth : cth + 1],
                    op0=Alu.is_equal, in1=SCR2[:], op1=Alu.mult,
                    accum_out=EACC[:, cth : cth + 1],
                )
            RANK = sb.tile([128, 4], dt.float32, tag="RANK")
            nc.vector.tensor_tensor(out=RANK[:], in0=GACC[:], in1=EACC[:], op=Alu.add)
            RKI = sb.tile([128, 4], dt.int32, tag="RKI")
            nc.vector.tensor_copy(RKI[:], RANK[:])

            # ---- scatter packed (py*1024+px) by rank ----
            PKD = sb.tile([128, 4], dt.float32, tag="PKD")
            nc.vector.scalar_tensor_tensor(
                out=PKD[:], in0=PY[:], scalar=1024.0, op0=Alu.mult, in1=PX[:],
                op1=Alu.add,
            )
            zt = sb.tile([1, NSLOT], dt.float32, tag="f3")
            nc.vector.memset(zt[:], 0.0)
            nc.sync.dma_start(kpd[:].rearrange("(o n) -> o n", o=1), zt[:])
            kview = kpd[:].rearrange("(n o) -> n o", o=1)
            for s in range(4):
                nc.gpsimd.indirect_dma_start(
                    out=kview,
                    out_offset=bass.IndirectOffsetOnAxis(
                        ap=RKI[:, s : s + 1], axis=0
                    ),
                    in_=PKD[:, s : s + 1],
                    in_offset=None,
                    bounds_check=TOPK - 1, oob_is_err=False,
                )

            # ---- readback + decode keypoints (k = 128*s + p) ----
            KP = sb.tile([128, 4], dt.float32, tag="KP")
            nc.sync.dma_start(KP[:], kpd[:].rearrange("(s p) -> p s", p=128))
            KY = sb.tile([128, 4], dt.float32, tag="KY")
            KX = sb.tile([128, 4], dt.float32, tag="KX")
            KPI = sb.tile([128, 4], dt.int32, tag="KPI")
            KYI = sb.tile([128, 4], dt.int32, tag="KYI")
            nc.vector.tensor_copy(KPI[:], KP[:])  # exact integer, any rounding
            nc.vector.tensor_scalar(
                out=KYI[:], in0=KPI[:], scalar1=10, scalar2=None,
                op0=Alu.logical_shift_right,
            )
            nc.vector.tensor_copy(KY[:], KYI[:])
            nc.vector.tensor_scalar(
                out=KYI[:], in0=KPI[:], scalar1=1023, scalar2=None,
                op0=Alu.bitwise_and,
            )
            nc.vector.tensor_copy(KX[:], KYI[:])

            # ---- patch spans (2565 elems each) + softmax refinement ----
            TB5 = sb.tile([128, 4], dt.float32, tag="TB5")
            nc.vector.scalar_tensor_tensor(
                out=TB5[:], in0=KY[:], scalar=640.0, op0=Alu.mult, in1=KX[:],
                op1=Alu.add,
            )
            nc.vector.tensor_scalar(
                out=TB5[:], in0=TB5[:], scalar1=-1282.0, scalar2=0.0,
                op0=Alu.add, op1=Alu.max,
            )
            TB5I = sb.tile([128, 4], dt.int32, tag="TB5I")
            nc.vector.tensor_copy(TB5I[:], TB5[:])
            P25 = sb.tile([128, 4, 25], dt.float32, tag="P25")
            sview = scores[:].rearrange("h w -> (h w)").rearrange("(n o) -> n o", o=1)
            RIDX = sb.tile([128, 4], dt.int32, tag="RIDX")
            for r in range(5):
                nc.vector.tensor_scalar(
                    out=RIDX[:], in0=TB5I[:], scalar1=640 * r, scalar2=None,
                    op0=Alu.add,
                )
                for s in range(4):
                    nc.gpsimd.indirect_dma_start(
                        out=P25[:, s, 5 * r : 5 * r + 5], out_offset=None, in_=sview,
                        in_offset=bass.IndirectOffsetOnAxis(
                            ap=RIDX[:, s : s + 1], axis=0
                        ),
                    )
            MAXV = sb.tile([128, 4], dt.float32, tag="MAXV")
            nc.vector.tensor_reduce(out=MAXV[:], in_=P25[:], axis=Ax.X, op=Alu.max)
            XE = sb.tile([128, 4, 25], dt.float32, tag="XE")
            nc.vector.tensor_tensor(
                out=XE[:], in0=P25[:], in1=MAXV[:].to_broadcast([128, 4, 25]),
                op=Alu.subtract,
            )
            nc.scalar.activation(
                out=XE[:].rearrange("p c k -> p (c k)"),
                in_=XE[:].rearrange("p c k -> p (c k)"),
                func=ActF.Exp, scale=10.0,
            )
            SSUM = sb.tile([128, 4], dt.float32, tag="SSUM")
            nc.vector.tensor_reduce(out=SSUM[:], in_=XE[:], axis=Ax.X, op=Alu.add)
            REC = sb.tile([128, 4], dt.float32, tag="REC")
            nc.vector.tensor_scalar(
                out=REC[:], in0=SSUM[:], scalar1=1e-12, scalar2=None, op0=Alu.add
            )
            nc.vector.reciprocal(out=REC[:], in_=REC[:])
            dxr = sb.tile([128, 100], dt.float32, tag="dxr")
            dyr = sb.tile([128, 100], dt.float32, tag="dyr")
            nc.sync.dma_start(dxr[:], cst["dxr"][:])
            nc.sync.dma_start(dyr[:], cst["dyr"][:])
            T100 = sb.tile([128, 4, 25], dt.float32, tag="T100")
            XR = sb.tile([128, 4], dt.float32, tag="XR")
            YR = sb.tile([128, 4], dt.float32, tag="YR")
            for ramp, out_t in ((dxr, XR), (dyr, YR)):
                nc.vector.tensor_tensor(
                    out=T100[:], in0=XE[:],
                    in1=ramp[:].rearrange("p (c k) -> p c k", c=4), op=Alu.mult,
                )
                nc.vector.tensor_reduce(
                    out=out_t[:], in_=T100[:], axis=Ax.X, op=Alu.add
                )
                nc.vector.tensor_tensor(
                    out=out_t[:], in0=out_t[:], in1=REC[:], op=Alu.mult
                )
            D2 = sb.tile([128, 4, 25], dt.float32, tag="D2")
            nc.vector.tensor_tensor(
                out=D2[:], in0=dxr[:].rearrange("p (c k) -> p c k", c=4),
                in1=XR[:].to_broadcast([128, 4, 25]), op=Alu.subtract,
            )
            nc.vector.tensor_scalar(
                out=D2[:], in0=D2[:], scalar1=0.5, scalar2=None, op0=Alu.mult
            )
            nc.vector.tensor_tensor(out=D2[:], in0=D2[:], in1=D2[:], op=Alu.mult)
            nc.vector.tensor_tensor(
                out=T100[:], in0=dyr[:].rearrange("p (c k) -> p c k", c=4),
                in1=YR[:].to_broadcast([128, 4, 25]), op=Alu.subtract,
            )
            nc.vector.tensor_scalar(
                out=T100[:], in0=T100[:], scalar1=0.5, scalar2=None, op0=Alu.mult
            )
            nc.vector.tensor_tensor(out=T100[:], in0=T100[:], in1=T100[:], op=Alu.mult)
            nc.vector.tensor_tensor(out=D2[:], in0=D2[:], in1=T100[:], op=Alu.add)
            nc.vector.tensor_tensor(out=D2[:], in0=D2[:], in1=XE[:], op=Alu.mult)
            DISP = sb.tile([128, 4], dt.float32, tag="DISP")
            nc.vector.tensor_reduce(out=DISP[:], in_=D2[:], axis=Ax.X, op=Alu.add)
            nc.vector.tensor_tensor(out=DISP[:], in0=DISP[:], in1=REC[:], op=Alu.mult)

            # ---- normalized kxy out ----
            KXA = sb.tile([128, 4], dt.float32, tag="KXA")
            KYA = sb.tile([128, 4], dt.float32, tag="KYA")
            nc.vector.tensor_tensor(out=KXA[:], in0=KX[:], in1=XR[:], op=Alu.add)
            nc.vector.tensor_tensor(out=KYA[:], in0=KY[:], in1=YR[:], op=Alu.add)
            KXN = sb.tile([128, 4], dt.float32, tag="KXN")
            KYN = sb.tile([128, 4], dt.float32, tag="KYN")
            nc.vector.tensor_scalar(
                out=KXN[:], in0=KXA[:], scalar1=float(2.0 / (W - 1)), scalar2=-1.0,
                op0=Alu.mult, op1=Alu.add,
            )
            nc.vector.tensor_scalar(
                out=KYN[:], in0=KYA[:], scalar1=float(2.0 / (H - 1)), scalar2=-1.0,
                op0=Alu.mult, op1=Alu.add,
            )
            KXY = sb.tile([128, 4, 2], dt.float32, tag="KXY")
            nc.vector.tensor_copy(
                KXY[:, :, 0:1], KXN[:].rearrange("p (c o) -> p c o", o=1)
            )
            nc.vector.tensor_copy(
                KXY[:, :, 1:2], KYN[:].rearrange("p (c o) -> p c o", o=1)
            )
            nc.sync.dma_start(
                o_kxy[0:384, :].rearrange("(s p) d -> p s d", p=128), KXY[:, 0:3, :]
            )
            nc.sync.dma_start(
                o_kxy[384:500, :].rearrange("(s p) d -> p s d", p=116),
                KXY[0:116, 3:4, :],
            )

            # ---- grid sample coords ----
            GX = sb.tile([128, 4], dt.float32, tag="GX")
            GY = sb.tile([128, 4], dt.float32, tag="GY")
            nc.vector.tensor_scalar(
                out=GX[:], in0=KXN[:], scalar1=1.0, scalar2=float(0.5 * (W - 1)),
                op0=Alu.add, op1=Alu.mult,
            )
            nc.vector.tensor_scalar(
                out=GX[:], in0=GX[:], scalar1=0.0, scalar2=float(W - 1),
                op0=Alu.max, op1=Alu.min,
            )
            nc.vector.tensor_scalar(
                out=GY[:], in0=KYN[:], scalar1=1.0, scalar2=float(0.5 * (H - 1)),
                op0=Alu.add, op1=Alu.mult,
            )
            nc.vector.tensor_scalar(
                out=GY[:], in0=GY[:], scalar1=0.0, scalar2=float(H - 1),
                op0=Alu.max, op1=Alu.min,
            )
            X0G = sb.tile([128, 4], dt.float32, tag="X0G")
            Y0G = sb.tile([128, 4], dt.float32, tag="Y0G")
            TI = sb.tile([128, 4], dt.int32, tag="TI")
            TF = sb.tile([128, 4], dt.float32, tag="TF")
            # floor robust to cast rounding mode: y = cast(x); y -= (y > x)
            for gsrc_t, gdst, hi in ((GX, X0G, W - 2), (GY, Y0G, H - 2)):
                nc.vector.tensor_copy(TI[:], gsrc_t[:])
                nc.vector.tensor_copy(gdst[:], TI[:])
                nc.vector.tensor_tensor(
                    out=TF[:], in0=gdst[:], in1=gsrc_t[:], op=Alu.is_gt
                )
                nc.vector.tensor_tensor(
                    out=gdst[:], in0=gdst[:], in1=TF[:], op=Alu.subtract
                )
                nc.vector.tensor_scalar(
                    out=gdst[:], in0=gdst[:], scalar1=0.0, scalar2=float(hi),
                    op0=Alu.max, op1=Alu.min,
                )
            WX = sb.tile([128, 4], dt.float32, tag="WX")
            WY = sb.tile([128, 4], dt.float32, tag="WY")
            nc.vector.tensor_tensor(out=WX[:], in0=GX[:], in1=X0G[:], op=Alu.subtract)
            nc.vector.tensor_tensor(out=WY[:], in0=GY[:], in1=Y0G[:], op=Alu.subtract)
            O00 = sb.tile([128, 4], dt.float32, tag="O00")
            nc.vector.scalar_tensor_tensor(
                out=O00[:], in0=Y0G[:], scalar=640.0, op0=Alu.mult, in1=X0G[:],
                op1=Alu.add,
            )
            O00I = sb.tile([128, 4], dt.int32, tag="O00I")
            nc.vector.tensor_copy(O00I[:], O00[:])

            # ---- kptscores: 4 span gathers (642 elems) + 2x2 extract ----
            SAB = sb.tile([128, 4, 2, 2], dt.float32, tag="SAB")
            OIDX = sb.tile([128, 4], dt.int32, tag="OIDX")
            for r in range(2):
                nc.vector.tensor_scalar(
                    out=OIDX[:], in0=O00I[:], scalar1=640 * r, scalar2=None,
                    op0=Alu.add,
                )
                for s in range(4):
                    nc.gpsimd.indirect_dma_start(
                        out=SAB[:, s, r, :], out_offset=None, in_=sview,
                        in_offset=bass.IndirectOffsetOnAxis(
                            ap=OIDX[:, s : s + 1], axis=0
                        ),
                    )
            TA = sb.tile([128, 4], dt.float32, tag="TA")
            TBv = sb.tile([128, 4], dt.float32, tag="TBv")
            KPT = sb.tile([128, 4], dt.float32, tag="KPT")
            nc.vector.tensor_tensor(
                out=TA[:], in0=SAB[:, :, 0, 1], in1=SAB[:, :, 0, 0], op=Alu.subtract
            )
            nc.vector.tensor_tensor(out=TA[:], in0=TA[:], in1=WX[:], op=Alu.mult)
            nc.vector.tensor_tensor(
                out=TA[:], in0=TA[:], in1=SAB[:, :, 0, 0], op=Alu.add
            )
            nc.vector.tensor_tensor(
                out=TBv[:], in0=SAB[:, :, 1, 1], in1=SAB[:, :, 1, 0], op=Alu.subtract
            )
            nc.vector.tensor_tensor(out=TBv[:], in0=TBv[:], in1=WX[:], op=Alu.mult)
            nc.vector.tensor_tensor(
                out=TBv[:], in0=TBv[:], in1=SAB[:, :, 1, 0], op=Alu.add
            )
            nc.vector.tensor_tensor(out=KPT[:], in0=TBv[:], in1=TA[:], op=Alu.subtract)
            nc.vector.tensor_tensor(out=KPT[:], in0=KPT[:], in1=WY[:], op=Alu.mult)
            nc.vector.tensor_tensor(out=KPT[:], in0=KPT[:], in1=TA[:], op=Alu.add)
            nc.sync.dma_start(
                o_kpt[0:384].rearrange("(s p) -> p s", p=128), KPT[:, 0:3]
            )
            nc.sync.dma_start(
                o_kpt[384:500].rearrange("(s p) -> p s", p=116), KPT[0:116, 3:4]
            )
            nc.sync.dma_start(
                o_disp[0:384].rearrange("(s p) -> p s", p=128), DISP[:, 0:3]
            )
            nc.sync.dma_start(
                o_disp[384:500].rearrange("(s p) -> p s", p=116), DISP[0:116, 3:4]
            )

            # ---- descriptor sampling from descT (keypoint-major) ----
            O00T = sb.tile([128, 4], dt.int32, tag="O00T")
            O10T = sb.tile([128, 4], dt.int32, tag="O10T")
            nc.vector.tensor_scalar(
                out=O10T[:], in0=O00I[:], scalar1=640, scalar2=None, op0=Alu.add
            )
            nc.vector.tensor_scalar(
                out=O00T[:], in0=O00I[:], scalar1=6, scalar2=None,
                op0=Alu.logical_shift_left,
            )
            nc.vector.tensor_scalar(
                out=O10T[:], in0=O10T[:], scalar1=6, scalar2=None,
                op0=Alu.logical_shift_left,
            )
            tflat = descT[:].rearrange("(n o) -> n o", o=1)
            DGA = sb.tile([128, 4, 128], dt.bfloat16, tag="DGA")
            DGB = sb.tile([128, 4, 128], dt.bfloat16, tag="DGB")
            for s in range(4):
                nc.gpsimd.indirect_dma_start(
                    out=DGA[:, s, :], out_offset=None, in_=tflat,
                    in_offset=bass.IndirectOffsetOnAxis(
                        ap=O00T[:, s : s + 1], axis=0
                    ),
                )
                nc.gpsimd.indirect_dma_start(
                    out=DGB[:, s, :], out_offset=None, in_=tflat,
                    in_offset=bass.IndirectOffsetOnAxis(
                        ap=O10T[:, s : s + 1], axis=0
                    ),
                )
            DA = sb.tile([128, 4, 64], dt.float32, tag="DA")
            DB = sb.tile([128, 4, 64], dt.float32, tag="DB")
            DSm = sb.tile([128, 4, 64], dt.float32, tag="DSm")
            nc.vector.tensor_tensor(
                out=DA[:], in0=DGA[:, :, 64:128], in1=DGA[:, :, 0:64], op=Alu.subtract
            )
            nc.vector.tensor_tensor(
                out=DA[:], in0=DA[:], in1=WX[:].to_broadcast([128, 4, 64]), op=Alu.mult
            )
            nc.vector.tensor_tensor(
                out=DA[:], in0=DA[:], in1=DGA[:, :, 0:64], op=Alu.add
            )
            nc.vector.tensor_tensor(
                out=DB[:], in0=DGB[:, :, 64:128], in1=DGB[:, :, 0:64], op=Alu.subtract
            )
            nc.vector.tensor_tensor(
                out=DB[:], in0=DB[:], in1=WX[:].to_broadcast([128, 4, 64]), op=Alu.mult
            )
            nc.vector.tensor_tensor(
                out=DB[:], in0=DB[:], in1=DGB[:, :, 0:64], op=Alu.add
            )
            nc.vector.tensor_tensor(out=DSm[:], in0=DB[:], in1=DA[:], op=Alu.subtract)
            nc.vector.tensor_tensor(
                out=DSm[:], in0=DSm[:], in1=WY[:].to_broadcast([128, 4, 64]),
                op=Alu.mult,
            )
            nc.vector.tensor_tensor(out=DSm[:], in0=DSm[:], in1=DA[:], op=Alu.add)
            DSQ = sb.tile([128, 4, 64], dt.float32, tag="DSQ")
            nc.vector.tensor_tensor(out=DSQ[:], in0=DSm[:], in1=DSm[:], op=Alu.mult)
            NRM = sb.tile([128, 4], dt.float32, tag="NRM")
            nc.vector.tensor_reduce(out=NRM[:], in_=DSQ[:], axis=Ax.X, op=Alu.add)
            nc.scalar.activation(out=NRM[:], in_=NRM[:], func=ActF.Sqrt)
            nc.vector.tensor_scalar(
                out=NRM[:], in0=NRM[:], scalar1=1e-12, scalar2=None, op0=Alu.max
            )
            nc.vector.reciprocal(out=NRM[:], in_=NRM[:])
            nc.vector.tensor_tensor(
                out=DSm[:], in0=DSm[:], in1=NRM[:].to_broadcast([128, 4, 64]),
                op=Alu.mult,
            )
            nc.sync.dma_start(
                o_desc[0:384, :].rearrange("(s p) d -> p s d", p=128), DSm[:, 0:3, :]
            )
            nc.sync.dma_start(
                o_desc[384:500, :].rearrange("(s p) d -> p s d", p=116),
                DSm[0:116, 3:4, :],
            )
    nc.finalize()
    return nc


_NC_CACHE = None


def _get_nc():
    global _NC_CACHE
    if _NC_CACHE is None:
        _NC_CACHE = build_nc()
    return _NC_CACHE


def kernel(scores_map, descriptor_map, sub_pixel=1, _want_time=False):
    scores_map = np.asarray(scores_map, dtype=np.float32)
    descriptor_map = np.asarray(descriptor_map, dtype=np.float32)
    B = scores_map.shape[0]
    nc = _get_nc()
    in_maps = []
    for b in range(B):
        m = {
            "scores": np.ascontiguousarray(scores_map[b, 0]),
            "descf": np.ascontiguousarray(descriptor_map[b].reshape(-1)),
        }
        for k, v in CONSTS.items():
            m[k] = v
        in_maps.append(m)
    try:
        res = run_bass_kernel_spmd(nc, in_maps, list(range(B)), trace=_want_time)
    except ModuleNotFoundError:
        res = run_bass_kernel_spmd(nc, in_maps, list(range(B)), trace=False)
    outs = res.results
    kxy = np.stack([o["out_kxy"] for o in outs])
    desc = np.stack([o["out_desc"] for o in outs])
    kpt = np.stack([o["out_kpt"] for o in outs])
    disp = np.stack([o["out_disp"] for o in outs])
    if _want_time:
        t = res.exec_time_ns
        if t is None:
            # no NTFF profiling hook in this environment: report the
            # cost-model timeline estimate for a single core instead
            from concourse.timeline_sim import TimelineSim

            t = int(TimelineSim(nc).simulate())
        return (kxy, desc, kpt, disp), t
    return kxy, desc, kpt, disp
